# revision 23
# baseline (speedup 1.0000x reference)
"""Trainium2 Bass kernel for nn_Attention_59691455480358 (sparse CLS attention).

Math: the reference computes softmax over
    logits[b, n] = (x[b,0]@W_q) . (x[b,1+n]@W_k) * C^-0.5,  n in [0, 2048).
Only the CLS query row matters and V is unused, so fold the K-projection into
the query side:

    t[b]        = W_k @ (x[b,0,:] @ W_q)          # [C] per example
    logits[b,n] = x[b,1+n,:] . t[b]               # row dot-products
    out[b]      = softmax(logits[b] * C^-0.5)

Sharding: pure data parallel - batch 16 over 8 NeuronCores (2 examples/core),
weights replicated (a ReduceScatter-based weight-sharded variant measured a
~69us collective latency in this environment - not viable).

The heavy pass (row dot products, 4.2M MACs/core) runs on the TensorEngine:
x ships HOST-TRANSPOSED per example (xT[c, n], bf16) so the PE contracts over
c on the partition dim: lhsT = one [128,1] column of tT (per-example t,
PE-transposed), rhs = [128, 512] slices of xT, accumulating a [1, 2048]
logit row per example in PSUM (4 banks) over the 8 c-chunks.  64 matmuls
~= 14us on PE, paced by the x DMA arrival.

DMA plan: all big inputs arrive host-shuffled partition-major so every
transfer reads 8-16KB contiguous per partition.  ONE queue (SP HWDGE)
carries W_q then W_kt as 2x1MB transfers each (the q/t matmuls pipeline
under them), then x: example 0 as two 2MB groups, example 1 as
2MB+1MB+512KB then two 256KB n-halves of the final c-chunk, so the last
matmuls and the first exp half overlap the final transfers.  A short bf16
PE warmup keeps HAM at full clock.  The ACT HWDGE queue carries the
tiny x0T and the output rows.  All PSUM flows through one 2-slot pool
(2 x 4 banks) so the two examples' logit rows coexist - example 1's matmuls
never wait on example 0's softmax.  Softmax per example: single ACT exp over
the [1, 2048] PSUM row (fused total-sum accumulator), DVE reciprocal, 1/S
multiply split DVE/ACT half-half, two 4KB output DMAs as halves complete.
No max-subtraction (scaled logits are ~N(0,1)).
"""
import sys

for _p in ("/opt/trn_rl_repo", "/root/.axon_site", "/root/.axon_site/_ro/trn_rl_repo",
           "/root/.axon_site/_ro/pypackages"):
    if _p not in sys.path:
        sys.path.append(_p)

from contextlib import ExitStack

import ml_dtypes
import numpy as np

import concourse.bass as bass  # noqa: F401
import concourse.tile as tile
from concourse import bacc, mybir
from concourse import bass_utils
from concourse.bass_interp import get_hw_module
from concourse.masks import make_identity

N_CORES = 8
B, N, C = 16, 2049, 1024
B_LOC = B // N_CORES        # 2 examples per core
P = 128                     # SBUF partitions / c-chunk size
CT = C // P                 # 8 c-chunks
NR = N - 1                  # 2048 key rows per example
FT = 4                      # 512-logit f-tiles (PSUM banks) per example
F = NR // FT                # 512
WG = 4                      # c-chunks per weight DMA (1MB transfers)
F32 = mybir.dt.float32
BF16 = mybir.dt.bfloat16
NP_BF16 = ml_dtypes.bfloat16


def build_nc():
    nc = bacc.Bacc("TRN2", target_bir_lowering=False, debug=False,
                   enable_asserts=True, num_devices=N_CORES)

    # all big inputs arrive HOST-SHUFFLED partition-major ([p, chunk, ...])
    # so every DMA reads 8-16KB contiguous per partition
    xt_d = nc.dram_tensor("xt", [B_LOC, P, CT, NR], BF16, kind="ExternalInput").ap()
    x0t_d = nc.dram_tensor("x0t", [P, CT * B_LOC], BF16, kind="ExternalInput").ap()
    wq_d = nc.dram_tensor("wq", [P, CT, C], BF16, kind="ExternalInput").ap()
    wkt_d = nc.dram_tensor("wkt", [P, CT, C], BF16, kind="ExternalInput").ap()
    o_d = nc.dram_tensor("o", [B_LOC, NR], F32, kind="ExternalOutput").ap()

    with tile.TileContext(nc) as tc, ExitStack() as ctx:
        sing = ctx.enter_context(tc.tile_pool(name="sing", bufs=1))
        xp = ctx.enter_context(tc.tile_pool(name="xp", bufs=1))
        # single rotating PSUM pool: 2 slots x 4 banks (slot sized by the
        # [1, 2048] logit rows; the small t-chain tiles rotate through too)
        pss = ctx.enter_context(tc.tile_pool(name="pss", bufs=2, space="PSUM"))

        ident = sing.tile([P, P], F32, tag="ident")
        make_identity(nc, ident[:])
        # PE warmup: ~20 cheap bf16 matmuls (alternating PSUM banks) keep the
        # TensorEngine active from ~t=5us so HAM un-throttles to 2.4GHz
        # before the real t-chain matmuls; results are discarded.
        warm_src = sing.tile([P, F], BF16, tag="warm_src")
        nc.gpsimd.memset(warm_src[:], 1.0)
        ps_w = [pss.tile([1, F], F32, tag="ps", name=f"ps_warm{k}")
                for k in range(2)]
        for k in range(12):
            nc.tensor.matmul(ps_w[k % 2][:], warm_src[:, 0:1], warm_src[:],
                             start=True, stop=True, skip_group_check=True)

        # --- x0T (tiny) on the ACT queue ------------------------------------
        x0t = sing.tile([P, CT * B_LOC], BF16, tag="x0t")
        nc.scalar.dma_start(x0t[:], x0t_d)

        # --- SP queue, priority order: wq, wkt (1MB groups), then x ---------
        # wq_sb cols [1024j:1024j+1024] = W_q rows-chunk j; same for wkt.
        wq_sb = sing.tile([P, CT * C], BF16, tag="wq")
        wkt_sb = sing.tile([P, CT * C], BF16, tag="wkt")
        for w_sb, w_d in ((wq_sb, wq_d), (wkt_sb, wkt_d)):
            for g in range(2):
                nc.sync.dma_start(
                    w_sb[:, C * WG * g:C * WG * (g + 1)]
                    .rearrange("p (j m) -> p j m", j=WG),
                    w_d[:, WG * g:WG * (g + 1), :])

        # x stream: example 0 as two 2MB groups (fewest transfers; its tail
        # is hidden mid-stream), example 1 as 2MB+1MB+512KB+2x256KB so its
        # final matmuls and softmax overlap the last transfers
        xts = {}           # (e, ci) -> (tile, j-index within tile)
        for g in range(2):
            xt_t = xp.tile([P, 4, NR], BF16, tag=f"x0_{g}", name=f"x0_{g}")
            nc.sync.dma_start(xt_t[:], xt_d[0, :, 4 * g:4 * (g + 1), :])
            for j in range(4):
                xts[(0, 4 * g + j)] = (xt_t, j)
        e1 = B_LOC - 1
        xt_a = xp.tile([P, 4, NR], BF16, tag="x1_a", name="x1_a")
        nc.sync.dma_start(xt_a[:], xt_d[e1, :, 0:4, :])
        for j in range(4):
            xts[(e1, j)] = (xt_a, j)
        xt_b = xp.tile([P, 2, NR], BF16, tag="x1_b", name="x1_b")
        nc.sync.dma_start(xt_b[:], xt_d[e1, :, 4:6, :])
        xts[(e1, 4)] = (xt_b, 0)
        xts[(e1, 5)] = (xt_b, 1)
        xt_c = xp.tile([P, 1, NR], BF16, tag="x1_c", name="x1_c")
        nc.sync.dma_start(xt_c[:], xt_d[e1, :, 6:7, :])
        xts[(e1, 6)] = (xt_c, 0)
        # very last chunk arrives as n[0:1024], n[1024:1536], n[1536:2048]
        # pieces so each f-tile's matmul and exp overlap the closing DMAs
        xl = {}
        for k, (n0, n1) in enumerate(((0, 1024), (1024, 1536), (1536, 2048))):
            xt_t = xp.tile([P, 1, n1 - n0], BF16, tag=f"xlast{k}",
                           name=f"xlast{k}")
            nc.sync.dma_start(
                xt_t[:], xt_d[B_LOC - 1, :, CT - 1:CT, n0:n1])
            xl[k] = xt_t

        # --- t chain: q = x0 @ Wq, t = Wk @ q, both [2, 1024] ---------------
        q_sb = sing.tile([B_LOC, C], F32, tag="q_sb")
        psq = [pss.tile([B_LOC, F], F32, tag="ps", name=f"psq{h}") for h in range(2)]
        for j in range(CT):
            for h in range(2):
                nc.tensor.matmul(psq[h][:], x0t[:, B_LOC * j:B_LOC * (j + 1)],
                                 wq_sb[:, C * j + F * h:C * j + F * (h + 1)],
                                 start=(j == 0), stop=(j == CT - 1))
        for h in range(2):
            nc.scalar.copy(q_sb[:, F * h:F * (h + 1)], psq[h][:])
        qt_sb = sing.tile([P, B_LOC * CT], BF16, tag="qT")
        for m in range(CT):
            ps = pss.tile([P, B_LOC], F32, tag="ps", name=f"psqt{m}")
            nc.tensor.transpose(ps[:], q_sb[:, P * m:P * (m + 1)],
                                ident[:B_LOC, :B_LOC])
            if m % 2 == 0:
                nc.scalar.copy(qt_sb[:, B_LOC * m:B_LOC * (m + 1)], ps[:])
            else:
                nc.vector.tensor_copy(qt_sb[:, B_LOC * m:B_LOC * (m + 1)], ps[:])
        t_sb = sing.tile([B_LOC, C], F32, tag="t_sb")
        ps_t = [pss.tile([B_LOC, F], F32, tag="ps", name=f"ps_t{h}") for h in range(2)]
        for m in range(CT):
            for h in range(2):
                nc.tensor.matmul(ps_t[h][:], qt_sb[:, B_LOC * m:B_LOC * (m + 1)],
                                 wkt_sb[:, C * m + F * h:C * m + F * (h + 1)],
                                 start=(m == 0), stop=(m == CT - 1))
        for h in range(2):
            nc.scalar.copy(t_sb[:, F * h:F * (h + 1)], ps_t[h][:])

        # --- tT [128, 2*8]: column 2m+e = c-chunk m of example e's t --------
        tt_sb = sing.tile([P, B_LOC * CT], BF16, tag="tT")
        for m in range(CT):
            ps = pss.tile([P, B_LOC], F32, tag="ps", name=f"pstt{m}")
            nc.tensor.transpose(ps[:], t_sb[:, P * m:P * (m + 1)],
                                ident[:B_LOC, :B_LOC])
            if m % 2 == 0:
                nc.scalar.copy(tt_sb[:, B_LOC * m:B_LOC * (m + 1)], ps[:])
            else:
                nc.vector.tensor_copy(tt_sb[:, B_LOC * m:B_LOC * (m + 1)], ps[:])

        # --- heavy pass: 64 PE matmuls into [1, 2048] PSUM rows + softmax ---
        inv_sqrt_c = float(C ** -0.5)
        for e in range(B_LOC):
            ps_l = pss.tile([1, NR], F32, tag="ps", name=f"L{e}")
            for ci in range(CT):
                for f in range(FT):
                    if e == B_LOC - 1 and ci == CT - 1:
                        if f < 2:
                            rhs = xl[0][:, 0, F * f:F * (f + 1)]
                        else:
                            rhs = xl[f - 1][:, 0, :]
                    else:
                        xt_t, j = xts[(e, ci)]
                        rhs = xt_t[:, j, F * f:F * (f + 1)]
                    nc.tensor.matmul(
                        ps_l[:, F * f:F * (f + 1)],
                        tt_sb[:, B_LOC * ci + e:B_LOC * ci + e + 1],
                        rhs, start=(ci == 0), stop=(ci == CT - 1))
            # exp in two halves: the first runs while the f2/f3 matmuls of
            # the last c-chunk are still on the PE
            ex = sing.tile([1, NR], F32, tag=f"E{e}", name=f"E{e}")
            sduo = sing.tile([1, 3], F32, tag=f"Sd{e}", name=f"Sd{e}")
            for k, (n0, n1) in enumerate(((0, 1024), (1024, 1536),
                                          (1536, 2048))):
                nc.scalar.activation(ex[:, n0:n1], ps_l[:, n0:n1],
                                     mybir.ActivationFunctionType.Exp,
                                     bias=0.0, scale=inv_sqrt_c,
                                     accum_out=sduo[:, k:k + 1])
            stot = sing.tile([1, 1], F32, tag=f"St{e}", name=f"St{e}")
            nc.vector.tensor_reduce(stot[:], sduo[:], axis=mybir.AxisListType.X,
                                    op=mybir.AluOpType.add)
            rv = sing.tile([1, 1], F32, tag=f"R{e}", name=f"R{e}")
            nc.vector.reciprocal(rv[:], stot[:])
            ot = sing.tile([1, NR], F32, tag=f"O{e}", name=f"O{e}")
            sp = 1344   # DVE (faster) takes the bigger piece
            nc.vector.tensor_scalar_mul(ot[:, :sp], ex[:, :sp], rv[:])
            # ex1's first half rides the (by-then idle) SP queue in parallel
            # with the scalar-queue half; ex0 outputs mid-stream, where the
            # SP queue is still full of x transfers, so it stays on scalar.
            q0 = nc.sync if e == B_LOC - 1 else nc.scalar
            q0.dma_start(o_d[e:e + 1, :sp], ot[:, :sp])
            nc.scalar.mul(ot[:, sp:], ex[:, sp:], rv[:])
            nc.scalar.dma_start(o_d[e:e + 1, sp:], ot[:, sp:])

    nc.compile()
    nc.m = get_hw_module(nc.m)
    return nc


_NC_CACHE = {}


def _get_nc():
    if "nc" not in _NC_CACHE:
        _NC_CACHE["nc"] = build_nc()
    return _NC_CACHE["nc"]


def _prep_inputs(x, w_qkv):
    """Host-side shard/layout prep (bf16 cast, per-example transpose of x,
    weight transpose).  Returns the per-core input maps."""
    x = np.asarray(x, dtype=np.float32)
    w = np.asarray(w_qkv, dtype=np.float32)
    x_bf = x.astype(NP_BF16)
    # transposed + partition-major-shuffled key rows: [16, 128, 8, 2048],
    # [b, p, j, n] = x[b, 1+n, 128j+p]
    xt = np.ascontiguousarray(
        x_bf[:, 1:, :].reshape(B, NR, CT, P).transpose(0, 3, 2, 1))
    x0 = x_bf[:, 0, :]                                        # [16, 1024]
    # weights partition-major: [p, j, m] = W[128j+p, m]
    wq = np.ascontiguousarray(
        w[:, :C].reshape(CT, P, C).transpose(1, 0, 2)).astype(NP_BF16)
    wkt = np.ascontiguousarray(
        w[:, C:2 * C].T.reshape(CT, P, C).transpose(1, 0, 2)).astype(NP_BF16)

    in_maps = []
    for c in range(N_CORES):
        x0c = x0[c * B_LOC:(c + 1) * B_LOC]                   # [2, 1024]
        x0t = np.ascontiguousarray(
            x0c.T.reshape(CT, P, B_LOC).transpose(1, 0, 2).reshape(P, CT * B_LOC))
        in_maps.append({"xt": xt[c * B_LOC:(c + 1) * B_LOC],
                        "x0t": x0t, "wq": wq, "wkt": wkt})
    return in_maps


def _run(x, w_qkv, **kwargs):
    assert np.asarray(x).shape == (B, N, C)
    in_maps = _prep_inputs(x, w_qkv)
    nc = _get_nc()
    res = bass_utils.run_bass_kernel_spmd(nc, in_maps,
                                          core_ids=list(range(N_CORES)), **kwargs)
    out = np.concatenate([res.results[c]["o"] for c in range(N_CORES)], axis=0)
    return out, res


def kernel(x, w_qkv):
    out, _ = _run(x, w_qkv)
    return out


# revision 24
# speedup vs baseline: 1.0004x; 1.0004x over previous
"""Trainium2 Bass kernel for nn_Attention_59691455480358 (sparse CLS attention).

Math: the reference computes softmax over
    logits[b, n] = (x[b,0]@W_q) . (x[b,1+n]@W_k) * C^-0.5,  n in [0, 2048).
Only the CLS query row matters and V is unused, so fold the K-projection into
the query side:

    t[b]        = W_k @ (x[b,0,:] @ W_q)          # [C] per example
    logits[b,n] = x[b,1+n,:] . t[b]               # row dot-products
    out[b]      = softmax(logits[b] * C^-0.5)

Sharding: pure data parallel - batch 16 over 8 NeuronCores (2 examples/core),
weights replicated (a ReduceScatter-based weight-sharded variant measured a
~69us collective latency in this environment - not viable).

The heavy pass (row dot products, 4.2M MACs/core) runs on the TensorEngine:
x ships HOST-TRANSPOSED per example (xT[c, n], bf16) so the PE contracts over
c on the partition dim: lhsT = one [128,1] column of tT (per-example t,
PE-transposed), rhs = [128, 512] slices of xT, accumulating a [1, 2048]
logit row per example in PSUM (4 banks) over the 8 c-chunks.  64 matmuls
~= 14us on PE, paced by the x DMA arrival.

DMA plan: all big inputs arrive host-shuffled partition-major so every
transfer reads 8-16KB contiguous per partition.  ONE queue (SP HWDGE)
carries W_q then W_kt as 2x1MB transfers each (the q/t matmuls pipeline
under them), then x: example 0 as two 2MB groups, example 1 as
2MB+1MB+512KB then 256+128+128KB n-pieces of the final c-chunk, so the
last matmuls and two of the three exp pieces overlap the final transfers.  A short bf16
PE warmup keeps HAM at full clock.  The ACT HWDGE queue carries the
tiny x0T and the output rows.  All PSUM flows through one 2-slot pool
(2 x 4 banks) so the two examples' logit rows coexist - example 1's matmuls
never wait on example 0's softmax.  Softmax per example: single ACT exp over
the [1, 2048] PSUM row (fused total-sum accumulator), DVE reciprocal, 1/S
multiply split DVE/ACT half-half, two 4KB output DMAs as halves complete.
No max-subtraction (scaled logits are ~N(0,1)).
"""
import sys

for _p in ("/opt/trn_rl_repo", "/root/.axon_site", "/root/.axon_site/_ro/trn_rl_repo",
           "/root/.axon_site/_ro/pypackages"):
    if _p not in sys.path:
        sys.path.append(_p)

from contextlib import ExitStack

import ml_dtypes
import numpy as np

import concourse.bass as bass  # noqa: F401
import concourse.tile as tile
from concourse import bacc, mybir
from concourse import bass_utils
from concourse.bass_interp import get_hw_module
from concourse.masks import make_identity

N_CORES = 8
B, N, C = 16, 2049, 1024
B_LOC = B // N_CORES        # 2 examples per core
P = 128                     # SBUF partitions / c-chunk size
CT = C // P                 # 8 c-chunks
NR = N - 1                  # 2048 key rows per example
FT = 4                      # 512-logit f-tiles (PSUM banks) per example
F = NR // FT                # 512
WG = 4                      # c-chunks per weight DMA (1MB transfers)
F32 = mybir.dt.float32
BF16 = mybir.dt.bfloat16
NP_BF16 = ml_dtypes.bfloat16


def build_nc():
    nc = bacc.Bacc("TRN2", target_bir_lowering=False, debug=False,
                   enable_asserts=True, num_devices=N_CORES)

    # all big inputs arrive HOST-SHUFFLED partition-major ([p, chunk, ...])
    # so every DMA reads 8-16KB contiguous per partition
    xt_d = nc.dram_tensor("xt", [B_LOC, P, CT, NR], BF16, kind="ExternalInput").ap()
    x0t_d = nc.dram_tensor("x0t", [P, CT * B_LOC], BF16, kind="ExternalInput").ap()
    wq_d = nc.dram_tensor("wq", [P, CT, C], BF16, kind="ExternalInput").ap()
    wkt_d = nc.dram_tensor("wkt", [P, CT, C], BF16, kind="ExternalInput").ap()
    o_d = nc.dram_tensor("o", [B_LOC, NR], F32, kind="ExternalOutput").ap()

    with tile.TileContext(nc) as tc, ExitStack() as ctx:
        sing = ctx.enter_context(tc.tile_pool(name="sing", bufs=1))
        xp = ctx.enter_context(tc.tile_pool(name="xp", bufs=1))
        # single rotating PSUM pool: 2 slots x 4 banks (slot sized by the
        # [1, 2048] logit rows; the small t-chain tiles rotate through too)
        pss = ctx.enter_context(tc.tile_pool(name="pss", bufs=2, space="PSUM"))

        ident = sing.tile([P, P], F32, tag="ident")
        make_identity(nc, ident[:])
        # PE warmup: ~20 cheap bf16 matmuls (alternating PSUM banks) keep the
        # TensorEngine active from ~t=5us so HAM un-throttles to 2.4GHz
        # before the real t-chain matmuls; results are discarded.
        warm_src = sing.tile([P, F], BF16, tag="warm_src")
        nc.gpsimd.memset(warm_src[:], 1.0)
        ps_w = [pss.tile([1, F], F32, tag="ps", name=f"ps_warm{k}")
                for k in range(2)]
        for k in range(12):
            nc.tensor.matmul(ps_w[k % 2][:], warm_src[:, 0:1], warm_src[:],
                             start=True, stop=True, skip_group_check=True)

        # --- x0T (tiny) on the ACT queue ------------------------------------
        x0t = sing.tile([P, CT * B_LOC], BF16, tag="x0t")
        nc.scalar.dma_start(x0t[:], x0t_d)

        # --- SP queue, priority order: wq, wkt (1MB groups), then x ---------
        # wq_sb cols [1024j:1024j+1024] = W_q rows-chunk j; same for wkt.
        wq_sb = sing.tile([P, CT * C], BF16, tag="wq")
        wkt_sb = sing.tile([P, CT * C], BF16, tag="wkt")
        for w_sb, w_d in ((wq_sb, wq_d), (wkt_sb, wkt_d)):
            for g in range(2):
                nc.sync.dma_start(
                    w_sb[:, C * WG * g:C * WG * (g + 1)]
                    .rearrange("p (j m) -> p j m", j=WG),
                    w_d[:, WG * g:WG * (g + 1), :])

        # x stream: example 0 as two 2MB groups (fewest transfers; its tail
        # is hidden mid-stream), example 1 as 2MB+1MB+512KB+2x256KB so its
        # final matmuls and softmax overlap the last transfers
        xts = {}           # (e, ci) -> (tile, j-index within tile)
        for g in range(2):
            xt_t = xp.tile([P, 4, NR], BF16, tag=f"x0_{g}", name=f"x0_{g}")
            nc.sync.dma_start(xt_t[:], xt_d[0, :, 4 * g:4 * (g + 1), :])
            for j in range(4):
                xts[(0, 4 * g + j)] = (xt_t, j)
        e1 = B_LOC - 1
        xt_a = xp.tile([P, 4, NR], BF16, tag="x1_a", name="x1_a")
        nc.sync.dma_start(xt_a[:], xt_d[e1, :, 0:4, :])
        for j in range(4):
            xts[(e1, j)] = (xt_a, j)
        xt_b = xp.tile([P, 2, NR], BF16, tag="x1_b", name="x1_b")
        nc.sync.dma_start(xt_b[:], xt_d[e1, :, 4:6, :])
        xts[(e1, 4)] = (xt_b, 0)
        xts[(e1, 5)] = (xt_b, 1)
        xt_c = xp.tile([P, 1, NR], BF16, tag="x1_c", name="x1_c")
        nc.sync.dma_start(xt_c[:], xt_d[e1, :, 6:7, :])
        xts[(e1, 6)] = (xt_c, 0)
        # very last chunk arrives as n[0:1024], n[1024:1536], n[1536:2048]
        # pieces so each f-tile's matmul and exp overlap the closing DMAs
        xl = {}
        for k, (n0, n1) in enumerate(((0, 1024), (1024, 1536), (1536, 2048))):
            xt_t = xp.tile([P, 1, n1 - n0], BF16, tag=f"xlast{k}",
                           name=f"xlast{k}")
            nc.sync.dma_start(
                xt_t[:], xt_d[B_LOC - 1, :, CT - 1:CT, n0:n1])
            xl[k] = xt_t

        # --- t chain: q = x0 @ Wq, t = Wk @ q, both [2, 1024] ---------------
        q_sb = sing.tile([B_LOC, C], F32, tag="q_sb")
        psq = [pss.tile([B_LOC, F], F32, tag="ps", name=f"psq{h}") for h in range(2)]
        for j in range(CT):
            for h in range(2):
                nc.tensor.matmul(psq[h][:], x0t[:, B_LOC * j:B_LOC * (j + 1)],
                                 wq_sb[:, C * j + F * h:C * j + F * (h + 1)],
                                 start=(j == 0), stop=(j == CT - 1))
        for h in range(2):
            nc.scalar.copy(q_sb[:, F * h:F * (h + 1)], psq[h][:])
        qt_sb = sing.tile([P, B_LOC * CT], BF16, tag="qT")
        for m in range(CT):
            ps = pss.tile([P, B_LOC], F32, tag="ps", name=f"psqt{m}")
            nc.tensor.transpose(ps[:], q_sb[:, P * m:P * (m + 1)],
                                ident[:B_LOC, :B_LOC])
            if m % 2 == 0:
                nc.scalar.copy(qt_sb[:, B_LOC * m:B_LOC * (m + 1)], ps[:])
            else:
                nc.vector.tensor_copy(qt_sb[:, B_LOC * m:B_LOC * (m + 1)], ps[:])
        t_sb = sing.tile([B_LOC, C], F32, tag="t_sb")
        ps_t = [pss.tile([B_LOC, F], F32, tag="ps", name=f"ps_t{h}") for h in range(2)]
        for m in range(CT):
            for h in range(2):
                nc.tensor.matmul(ps_t[h][:], qt_sb[:, B_LOC * m:B_LOC * (m + 1)],
                                 wkt_sb[:, C * m + F * h:C * m + F * (h + 1)],
                                 start=(m == 0), stop=(m == CT - 1))
        for h in range(2):
            nc.scalar.copy(t_sb[:, F * h:F * (h + 1)], ps_t[h][:])

        # --- tT [128, 2*8]: column 2m+e = c-chunk m of example e's t --------
        tt_sb = sing.tile([P, B_LOC * CT], BF16, tag="tT")
        for m in range(CT):
            ps = pss.tile([P, B_LOC], F32, tag="ps", name=f"pstt{m}")
            nc.tensor.transpose(ps[:], t_sb[:, P * m:P * (m + 1)],
                                ident[:B_LOC, :B_LOC])
            if m % 2 == 0:
                nc.scalar.copy(tt_sb[:, B_LOC * m:B_LOC * (m + 1)], ps[:])
            else:
                nc.vector.tensor_copy(tt_sb[:, B_LOC * m:B_LOC * (m + 1)], ps[:])

        # --- heavy pass: 64 PE matmuls into [1, 2048] PSUM rows + softmax ---
        inv_sqrt_c = float(C ** -0.5)
        for e in range(B_LOC):
            ps_l = pss.tile([1, NR], F32, tag="ps", name=f"L{e}")
            for ci in range(CT):
                for f in range(FT):
                    if e == B_LOC - 1 and ci == CT - 1:
                        if f < 2:
                            rhs = xl[0][:, 0, F * f:F * (f + 1)]
                        else:
                            rhs = xl[f - 1][:, 0, :]
                    else:
                        xt_t, j = xts[(e, ci)]
                        rhs = xt_t[:, j, F * f:F * (f + 1)]
                    nc.tensor.matmul(
                        ps_l[:, F * f:F * (f + 1)],
                        tt_sb[:, B_LOC * ci + e:B_LOC * ci + e + 1],
                        rhs, start=(ci == 0), stop=(ci == CT - 1))
            # exp in two halves: the first runs while the f2/f3 matmuls of
            # the last c-chunk are still on the PE
            ex = sing.tile([1, NR], F32, tag=f"E{e}", name=f"E{e}")
            sduo = sing.tile([1, 3], F32, tag=f"Sd{e}", name=f"Sd{e}")
            for k, (n0, n1) in enumerate(((0, 1024), (1024, 1536),
                                          (1536, 2048))):
                nc.scalar.activation(ex[:, n0:n1], ps_l[:, n0:n1],
                                     mybir.ActivationFunctionType.Exp,
                                     bias=0.0, scale=inv_sqrt_c,
                                     accum_out=sduo[:, k:k + 1])
            stot = sing.tile([1, 1], F32, tag=f"St{e}", name=f"St{e}")
            nc.vector.tensor_reduce(stot[:], sduo[:], axis=mybir.AxisListType.X,
                                    op=mybir.AluOpType.add)
            rv = sing.tile([1, 1], F32, tag=f"R{e}", name=f"R{e}")
            nc.vector.reciprocal(rv[:], stot[:])
            ot = sing.tile([1, NR], F32, tag=f"O{e}", name=f"O{e}")
            sp = 1344   # DVE (faster) takes the bigger piece
            nc.vector.tensor_scalar_mul(ot[:, :sp], ex[:, :sp], rv[:])
            # ex1's first half rides the (by-then idle) SP queue in parallel
            # with the scalar-queue half; ex0 outputs mid-stream, where the
            # SP queue is still full of x transfers, so it stays on scalar.
            q0 = nc.sync if e == B_LOC - 1 else nc.scalar
            q0.dma_start(o_d[e:e + 1, :sp], ot[:, :sp])
            nc.scalar.mul(ot[:, sp:], ex[:, sp:], rv[:])
            nc.scalar.dma_start(o_d[e:e + 1, sp:], ot[:, sp:])

    nc.compile()
    nc.m = get_hw_module(nc.m)
    return nc


_NC_CACHE = {}


def _get_nc():
    if "nc" not in _NC_CACHE:
        _NC_CACHE["nc"] = build_nc()
    return _NC_CACHE["nc"]


def _prep_inputs(x, w_qkv):
    """Host-side shard/layout prep (bf16 cast, per-example transpose of x,
    weight transpose).  Returns the per-core input maps."""
    x = np.asarray(x, dtype=np.float32)
    w = np.asarray(w_qkv, dtype=np.float32)
    x_bf = x.astype(NP_BF16)
    # transposed + partition-major-shuffled key rows: [16, 128, 8, 2048],
    # [b, p, j, n] = x[b, 1+n, 128j+p]
    xt = np.ascontiguousarray(
        x_bf[:, 1:, :].reshape(B, NR, CT, P).transpose(0, 3, 2, 1))
    x0 = x_bf[:, 0, :]                                        # [16, 1024]
    # weights partition-major: [p, j, m] = W[128j+p, m]
    wq = np.ascontiguousarray(
        w[:, :C].reshape(CT, P, C).transpose(1, 0, 2)).astype(NP_BF16)
    wkt = np.ascontiguousarray(
        w[:, C:2 * C].T.reshape(CT, P, C).transpose(1, 0, 2)).astype(NP_BF16)

    in_maps = []
    for c in range(N_CORES):
        x0c = x0[c * B_LOC:(c + 1) * B_LOC]                   # [2, 1024]
        x0t = np.ascontiguousarray(
            x0c.T.reshape(CT, P, B_LOC).transpose(1, 0, 2).reshape(P, CT * B_LOC))
        in_maps.append({"xt": xt[c * B_LOC:(c + 1) * B_LOC],
                        "x0t": x0t, "wq": wq, "wkt": wkt})
    return in_maps


def _run(x, w_qkv, **kwargs):
    assert np.asarray(x).shape == (B, N, C)
    in_maps = _prep_inputs(x, w_qkv)
    nc = _get_nc()
    res = bass_utils.run_bass_kernel_spmd(nc, in_maps,
                                          core_ids=list(range(N_CORES)), **kwargs)
    out = np.concatenate([res.results[c]["o"] for c in range(N_CORES)], axis=0)
    return out, res


def kernel(x, w_qkv):
    out, _ = _run(x, w_qkv)
    return out
